# revision 8
# baseline (speedup 1.0000x reference)
"""3-layer GATv2 (PyG GATv2Conv semantics) on 8 Trainium2 NeuronCores.

Distribution: nodes sharded 12500/core; edges (excl. appended self-loops)
partitioned by dst core, packed per (superblock, src%4 class) into 128-edge
gather groups that may straddle the superblock's dst blocks.

Layer 1 phase A ([xl|xr] = x @ [Wl|Wr]) runs chunked (14 blocks per DMA);
layers 2/3 fold their phase A into the previous layer's per-block tail (the
ELU output hTn is already feature-major = the matmul lhsT), so between
layers only the AllGather remains. Layer 3 all-gathers an xl-only table
(half the bytes).

Phase B per superblock: S matrices (is_equal against host-precomputed
block-shifted rel values, one per (slot, dst-block) pair), dst gathers
(local rows, 4 quarter calls), dense self-loop slots (one batched
dma_start), src gathers (4 src%4 class calls) -- all gathers spread across
the 4 SWDGE queues (each runs on its own Q7 core pair, pipelining ~4x).
z = xl[src]+xr[dst], lrelu on the Scalar engine (Prelu alpha=0.2, same act
table as Exp), score = reduce(z*a), w = exp(score), wlhs = [w*xg | w].
Per dst block: matmul S_pair.T @ wlhs accumulates weighted sums +
denominators in PSUM; divide, bias, ELU (min via tensor_tensor against a
zero tile -- tensor_scalar would block SWDGE on the shared SBUF port),
transpose, next-layer matmul, superblock-batched table writes. Layer 3:
divide then head-mean, f32 shard output, host concat.
"""
import sys
sys.path.insert(0, "/opt/trn_rl_repo")
import numpy as np
import ml_dtypes

N = 100000
E = 800000
NCORES = 8
SHARD = N // NCORES        # 12500
P = 128
NBLK = (SHARD + P - 1) // P  # 98
SB = 4                      # node blocks per superblock
CH = 14                     # layer-1 phase A blocks per chunk (98 = 7*14)
FIN = 64
H = 4
C1, C3 = 16, 32
F1 = H * C1                # 64
F3 = H * C3                # 128
NEG_SLOPE = 0.2
NQ = 4                      # SWDGE queues (Q7 core pairs)

BF16 = ml_dtypes.bfloat16

_cache = {}


class Meta:
    pass


def _preprocess(edge_index):
    """Sort real edges by dst; per (core, superblock, class) pack into
    128-edge groups (may straddle dst blocks). Appended self-loops become
    per-block dense slots. Pair list (slot, block) is the union over cores
    so the program is SPMD-uniform; a core without edges in a pair gets an
    all-miss rel row (S rows zero)."""
    src = edge_index[0]
    dst = edge_index[1]
    order = np.argsort(dst, kind="stable")
    src_s = src[order].astype(np.int64)
    dst_s = dst[order].astype(np.int64)

    core = dst_s // SHARD
    blk = (dst_s - core * SHARD) // P
    key = core * NBLK + blk
    cnt = np.bincount(key, minlength=NCORES * NBLK).reshape(NCORES, NBLK)
    starts = np.concatenate([[0], np.cumsum(cnt.reshape(-1))])

    NCLS = 4
    m = Meta()
    m.NCLS = NCLS
    m.NSB = (NBLK + SB - 1) // SB
    m.sb_blocks = [list(range(s * SB, min(NBLK, (s + 1) * SB))) for s in range(m.NSB)]

    # per (core, sb, class): concatenated (src, sb-rel dst) runs
    runs = {}
    gcnt = np.zeros((NCORES, m.NSB, NCLS), np.int64)
    for c in range(NCORES):
        for s, bs in enumerate(m.sb_blocks):
            sr_all, rel_all = [], []
            for b in bs:
                i = c * NBLK + b
                s0, e0 = starts[i], starts[i + 1]
                sr_all.append(src_s[s0:e0])
                rel_all.append(dst_s[s0:e0] - c * SHARD - bs[0] * P)
            sr_all = np.concatenate(sr_all)
            rel_all = np.concatenate(rel_all)
            cls = sr_all % NCLS
            for r in range(NCLS):
                sel = cls == r
                runs[(c, s, r)] = (sr_all[sel], rel_all[sel])
                gcnt[c, s, r] = sel.sum()
    Gsb = np.maximum(1, -(-gcnt.max(axis=0) // P))   # [NSB, NCLS] groups

    m.sb_cls_off = []   # per sb: class group offsets [o0..o4]
    m.sb_g = []         # per sb: gather slots
    m.sb_s = []         # per sb: total slots (gather + dense)
    for s, bs in enumerate(m.sb_blocks):
        offs = [0]
        for r in range(NCLS):
            offs.append(offs[-1] + int(Gsb[s, r]))
        m.sb_cls_off.append(offs)
        m.sb_g.append(offs[-1])
        m.sb_s.append(offs[-1] + len(bs))
    m.SLOTmax = max(m.sb_s)
    m.Gtot = sum(m.sb_g)
    sb_goff = np.concatenate([[0], np.cumsum(m.sb_g)])
    m.sb_goff = [int(v) for v in sb_goff]

    # fill per-core gather idx arrays + per-slot sb-rel dst (BIGNEG pad)
    BIGNEG = -16000.0
    src_idx = np.zeros((NCORES, m.Gtot * P), np.int16)
    dst_idx = np.zeros((NCORES, m.Gtot * P), np.int16)
    slot_rel = np.full((NCORES, m.Gtot, P), BIGNEG, np.float32)
    for c in range(NCORES):
        for s in range(m.NSB):
            gbase = m.sb_goff[s]
            offs = m.sb_cls_off[s]
            for r in range(NCLS):
                sr, rel = runs[(c, s, r)]
                o = (gbase + offs[r]) * P
                src_idx[c, o:o + len(sr)] = sr // NCLS
                dst_idx[c, o:o + len(rel)] = rel + m.sb_blocks[s][0] * P
                fl = slot_rel[c, gbase + offs[r]:gbase + offs[r + 1]].reshape(-1)
                fl[:len(rel)] = rel
    # pairs: union over cores of (slot, block_j) overlaps + dense pairs
    m.sb_pairs = []     # per sb: list of (slot, jj)
    m.blk_pairs = [[] for _ in range(NBLK)]  # per block: (pair_idx, slot_idx) sb-local
    m.sb_p = []         # per sb: pair count
    for s, bs in enumerate(m.sb_blocks):
        gbase = m.sb_goff[s]
        pairs = []
        for sl in range(m.sb_g[s]):
            jset = set()
            for c in range(NCORES):
                rels = slot_rel[c, gbase + sl]
                valid = rels > BIGNEG
                if valid.any():
                    jset |= set((rels[valid].astype(np.int64) // P).tolist())
            for jj in sorted(jset):
                pairs.append((sl, jj))
        for i, b in enumerate(bs):
            pairs.append((m.sb_g[s] + i, i))   # dense slot pair
        m.sb_pairs.append(pairs)
        m.sb_p.append(len(pairs))
        for pi, (sl, jj) in enumerate(pairs):
            m.blk_pairs[bs[0] + jj].append((pi, sl))
    m.NPRmax = max(m.sb_p)
    m.Ptot = sum(m.sb_p)
    sb_poff = np.concatenate([[0], np.cumsum(m.sb_p)])
    m.sb_poff = [int(v) for v in sb_poff]

    # per-pair shifted rel values (the S-build comparison targets)
    rel_pairs = np.full((NCORES, m.Ptot, P), -1.0, np.float32)
    for s, bs in enumerate(m.sb_blocks):
        gbase = m.sb_goff[s]
        pbase = m.sb_poff[s]
        for pi, (sl, jj) in enumerate(m.sb_pairs[s]):
            if sl < m.sb_g[s]:
                v = slot_rel[:, gbase + sl] - jj * P    # [NCORES, P]
                rel_pairs[:, pbase + pi] = np.where(v > BIGNEG / 2, v, -1.0)
            else:
                rel_pairs[:, pbase + pi] = np.arange(P)[None, :]

    # Wrap to dma_gather idx layout: flat[k] -> wrapped[k % 16, k // 16],
    # replicated to all 8 16-partition groups (each Q7 core pair reads its
    # own partitions, so all 4 SWDGE queues see the same idx stream).
    def wrap(a):
        w = a.reshape(NCORES, m.Gtot * P // 16, 16).transpose(0, 2, 1)
        return np.tile(w, (1, 8, 1)).copy()

    m.src_w = wrap(src_idx)
    m.dst_w = wrap(dst_idx)
    m.rel_pm = rel_pairs.transpose(0, 2, 1).astype(BF16).copy()  # [NC, P, Ptot]
    return m


def _build_program(m):
    import os
    import concourse.bass as bass
    import concourse.bacc as bacc
    import concourse.tile as tile
    from concourse import mybir, library_config

    STAGE = int(os.environ.get("BK_STAGE", "99"))
    NLAYER = int(os.environ.get("BK_NLAYER", "3"))
    SP = bool(int(os.environ.get("BK_SP", "0")))   # single_packet experiment

    bf16, f32, i16 = mybir.dt.bfloat16, mybir.dt.float32, mybir.dt.int16
    AF = mybir.ActivationFunctionType
    OP = mybir.AluOpType
    X = mybir.AxisListType.X

    nc = bacc.Bacc("TRN2", target_bir_lowering=False, num_swdge_queues=NQ)

    WI = m.Gtot * P // 16
    xT_d = nc.dram_tensor("xT", [FIN, SHARD], bf16, kind="ExternalInput")
    srcw_d = nc.dram_tensor("srcw", [P, WI], i16, kind="ExternalInput")
    dstw_d = nc.dram_tensor("dstw", [P, WI], i16, kind="ExternalInput")
    dstrel_d = nc.dram_tensor("dstrel", [P, m.Ptot], bf16, kind="ExternalInput")
    iota_d = nc.dram_tensor("iota", [P, m.NPRmax * P], bf16, kind="ExternalInput")
    W_d = [nc.dram_tensor(f"W{l}", [FIN, 2 * (F1 if l < 3 else F3)], bf16, kind="ExternalInput") for l in (1, 2, 3)]
    arep_d = [nc.dram_tensor(f"arep{l}", [P, m.SLOTmax * (F1 if l < 3 else F3)], bf16, kind="ExternalInput") for l in (1, 2, 3)]
    brep_d = [nc.dram_tensor("brep1", [P, F1], bf16, kind="ExternalInput"),
              nc.dram_tensor("brep2", [P, F1], bf16, kind="ExternalInput"),
              nc.dram_tensor("brep3", [P, C3], f32, kind="ExternalInput")]
    ident_d = nc.dram_tensor("ident", [P, P], bf16, kind="ExternalInput")
    out_d = nc.dram_tensor("out_shard", [SHARD, C3], f32, kind="ExternalOutput")

    def internal(name, shape, dt, shared=False):
        return nc.dram_tensor(name, shape, dt, kind="Internal",
                              addr_space="Shared" if shared else "Local")

    xlr_sh = [internal(f"xlrsh{l}", [SHARD, 2 * (F1 if l < 3 else F3)], bf16) for l in (1, 2, 3)]
    xl3_sh = internal("xl3sh", [SHARD, F3], bf16)
    xlr_cc = [internal(f"xlrcc{l}", [N, 2 * F1], bf16, shared=True) for l in (1, 2)]
    xl3_cc = internal("xl3cc", [N, F3], bf16, shared=True)

    RG = [list(range(NCORES))]

    with tile.TileContext(nc) as tc:
        nc.gpsimd.load_library(library_config.mlp)
        with tc.tile_pool(name="const", bufs=1) as cpool, \
             tc.tile_pool(name="work", bufs=2) as wpool, \
             tc.tile_pool(name="mmA", bufs=3) as apool, \
             tc.tile_pool(name="tail", bufs=2) as tpool, \
             tc.tile_pool(name="psA", bufs=2, space="PSUM") as ppA, \
             tc.tile_pool(name="psB", bufs=2, space="PSUM") as ppB, \
             tc.tile_pool(name="psT", bufs=2, space="PSUM") as ppT:

            srcw = cpool.tile([P, WI], i16)
            dstw = cpool.tile([P, WI], i16)
            dstrel = cpool.tile([P, m.Ptot], bf16)
            iota = cpool.tile([P, m.NPRmax * P], bf16)
            ident = cpool.tile([P, P], bf16)
            for t, d in [(srcw, srcw_d), (dstw, dstw_d), (dstrel, dstrel_d),
                         (iota, iota_d), (ident, ident_d)]:
                nc.sync.dma_start(t[:], d[:])
            zero64 = cpool.tile([P, F1], bf16, tag="zero64")
            nc.vector.memset(zero64[:], 0.0)
            W_sb, arep_sb, brep_sb = [], [], []
            for li in range(3):
                Fl = F1 if li < 2 else F3
                w = cpool.tile([FIN, 2 * Fl], bf16, tag=f"W{li}")
                nc.sync.dma_start(w[:], W_d[li][:])
                W_sb.append(w)
                a = cpool.tile([P, m.SLOTmax * Fl], bf16, tag=f"arep{li}")
                nc.sync.dma_start(a[:], arep_d[li][:])
                arep_sb.append(a)
                b = cpool.tile([P, F1 if li < 2 else C3], bf16 if li < 2 else f32, tag=f"brep{li}")
                nc.sync.dma_start(b[:], brep_d[li][:])
                brep_sb.append(b)

            # ---- layer 1 phase A: x @ [W1l|W1r], chunked ----
            FE0 = 2 * F1
            for c0 in range(0, NBLK, CH):
                nblk = min(CH, NBLK - c0)
                n0 = c0 * P
                ncols = min(SHARD - n0, nblk * P)
                hTc = apool.tile([FIN, CH * P], bf16, tag="hTc")
                nc.sync.dma_start(hTc[:, :ncols], xT_d[:, n0:n0 + ncols])
                xlrc = apool.tile([P, CH, FE0], bf16, tag="xlrc")
                for j in range(nblk):
                    mm = min(P, SHARD - (c0 + j) * P)
                    psA = ppA.tile([P, 2 * F3], f32, tag="psA", space="PSUM")
                    nc.tensor.matmul(psA[:mm, :FE0], lhsT=hTc[:, j * P:j * P + mm],
                                     rhs=W_sb[0][:], start=True, stop=True)
                    nc.scalar.copy(xlrc[:mm, j, :], psA[:mm, :FE0])
                nfull = nblk if (c0 + nblk) * P <= SHARD else nblk - 1
                if nfull:
                    nc.sync.dma_start(
                        xlr_sh[0][n0:n0 + nfull * P, :]
                        .rearrange("(j p) f -> p j f", p=P),
                        xlrc[:, :nfull, :])
                if nfull < nblk:
                    mm = SHARD - (c0 + nfull) * P
                    nc.sync.dma_start(xlr_sh[0][(c0 + nfull) * P:SHARD, :],
                                      xlrc[:mm, nfull, :])

            for li in range(3):
                l3 = (li == 2)
                Fl = F3 if l3 else F1
                Cl = C3 if l3 else C1
                FE = 2 * Fl            # row width of xlr tensors
                FEn = 2 * (F3 if li + 1 >= 2 else F1)   # next layer row width

                if STAGE >= 2:
                    if not l3:
                        nc.gpsimd.collective_compute(
                            "AllGather", mybir.AluOpType.bypass, replica_groups=RG,
                            ins=[xlr_sh[li][:]], outs=[xlr_cc[li][:]])
                        gtab = xlr_cc[li]
                    else:
                        nc.gpsimd.collective_compute(
                            "AllGather", mybir.AluOpType.bypass, replica_groups=RG,
                            ins=[xl3_sh[:]], outs=[xl3_cc[:]])
                        gtab = xl3_cc
                else:
                    gtab = xlr_sh[li] if not l3 else xl3_sh

                # gather source views: 4 src classes (src%4, idx=src//4)
                if not l3:
                    src_tabs = [gtab[r::4, :] for r in range(m.NCLS)]
                    dst_tab = xlr_sh[li][:]                   # local rows, use xr half
                    GELEM, GSTEP = FE, 4 * FE
                    DELEM, DSTEP = FE, FE
                else:
                    src_tabs = [gtab[r::4, :] for r in range(m.NCLS)]
                    dst_tab = xlr_sh[li][:, F3:2 * F3]        # xr3 half
                    GELEM, GSTEP = F3, 4 * F3
                    DELEM, DSTEP = F3, FE

                # ---- phase B ----
                if STAGE < 3 or li >= NLAYER:
                    continue
                for s in range(m.NSB):
                    SG = m.sb_g[s]       # gather slots
                    ST = m.sb_s[s]       # total slots (gather + dense)
                    NPR = m.sb_p[s]      # pairs
                    go = m.sb_goff[s]
                    po = m.sb_poff[s]
                    wo = go * P // 16
                    bs = m.sb_blocks[s]
                    nb = len(bs)
                    b0 = bs[0] * P
                    nbf = nb if b0 + nb * P <= SHARD else nb - 1  # full dense blocks
                    # S matrices first (no gather dependency; overlaps AG)
                    S = wpool.tile([P, m.NPRmax, P], bf16, tag="S")
                    nc.vector.tensor_tensor(
                        out=S[:, :NPR, :],
                        in0=iota[:, :NPR * P].rearrange("p (g n) -> p g n", g=NPR),
                        in1=dstrel[:, po:po + NPR].to_broadcast([P, NPR, P]),
                        op=OP.is_equal)
                    xg = wpool.tile([P, m.SLOTmax, F3 if l3 else FE], bf16, tag="xg")
                    xrg = wpool.tile([P, m.SLOTmax, F3 if l3 else FE], bf16, tag="xrg")
                    # dst gather (local table): split into NQ quarter-calls
                    qcuts = [SG * q // NQ for q in range(NQ + 1)]
                    for q in range(NQ):
                        qs, qe = qcuts[q], qcuts[q + 1]
                        if qe == qs:
                            continue
                        nc.gpsimd.dma_gather(
                            out_ap=xrg[:, qs:qe, :DELEM], in_ap=dst_tab,
                            idxs_ap=dstw[:, wo + qs * 8:wo + qe * 8],
                            num_idxs=(qe - qs) * P, num_idxs_reg=(qe - qs) * P,
                            elem_size=DELEM, elem_step=DSTEP,
                            single_packet=SP, queue_num=q)
                    # dense self-loop slots: sb's own rows, batched DMAs
                    if not l3:
                        dsrc = [(xg, slice(0, FE), slice(0, FE)),
                                (xrg, slice(0, FE), slice(0, FE))]
                    else:
                        dsrc = [(xg, slice(0, F3), slice(0, F3)),
                                (xrg, slice(0, F3), slice(F3, 2 * F3))]
                    for tl, tsl, csl in dsrc:
                        if nbf:
                            nc.sync.dma_start(
                                tl[:, SG:SG + nbf, tsl],
                                xlr_sh[li][b0:b0 + nbf * P, csl]
                                .rearrange("(j p) f -> p j f", p=P))
                        if nbf < nb:
                            mm = SHARD - (bs[0] + nbf) * P
                            nc.sync.dma_start(
                                tl[:mm, SG + nbf, tsl],
                                xlr_sh[li][(bs[0] + nbf) * P:SHARD, csl])
                    # src gathers (need the all-gathered table): class r -> queue r
                    offs = m.sb_cls_off[s]
                    for r in range(m.NCLS):
                        nr = (offs[r + 1] - offs[r]) * P
                        if nr == 0:
                            continue
                        nc.gpsimd.dma_gather(
                            out_ap=xg[:, offs[r]:offs[r + 1], :GELEM], in_ap=src_tabs[r],
                            idxs_ap=srcw[:, wo + offs[r] * 8:wo + offs[r + 1] * 8],
                            num_idxs=nr, num_idxs_reg=nr, elem_size=GELEM,
                            elem_step=GSTEP, single_packet=SP, queue_num=r)
                    if STAGE < 4:
                        continue
                    xgv = xg[:, :ST, :Fl]                      # xl[src]
                    xrv = xrg[:, :ST, Fl:FE] if not l3 else xrg[:, :ST, :Fl]  # xr[dst]
                    # z lives in the unused xl[dst] half of xrg (l1/2) or
                    # overwrites xr in place (l3): z = xg + xr, lrelu (ACT)
                    zv = xrg[:, :ST, :Fl]
                    nc.vector.tensor_tensor(out=zv, in0=xgv, in1=xrv, op=OP.add)
                    nc.scalar.activation(zv, zv, AF.Prelu, alpha=NEG_SLOPE)
                    nc.vector.tensor_tensor(
                        out=zv, in0=zv,
                        in1=arep_sb[li][:, :ST * Fl].rearrange("p (g f) -> p g f", g=ST),
                        op=OP.mult)
                    score = wpool.tile([P, m.SLOTmax * H], f32, tag="score")
                    nc.vector.tensor_reduce(
                        out=score[:, :ST * H],
                        in_=zv.rearrange("p g (h c) -> p g h c", h=H),
                        axis=X, op=OP.add)
                    wlhs = wpool.tile([P, m.SLOTmax, F3 + H], bf16, tag="wlhs")
                    nc.scalar.activation(
                        wlhs[:, :ST, Fl:Fl + H],
                        score[:, :ST * H].rearrange("p (g h) -> p g h", g=ST),
                        AF.Exp)
                    nc.vector.tensor_tensor(
                        out=wlhs[:, :ST, :Fl].rearrange("p g (h c) -> p g h c", h=H),
                        in0=xgv.rearrange("p g (h c) -> p g h c", h=H),
                        in1=wlhs[:, :ST, Fl:Fl + H, None].to_broadcast([P, ST, H, Cl]),
                        op=OP.mult)

                    if STAGE < 5:
                        continue
                    xlrn = None
                    if not l3:
                        xlrn = apool.tile([P, SB, 2 * F3], bf16, tag="xlrn")
                    for ib, b in enumerate(bs):
                        n0 = b * P
                        mm = min(P, SHARD - n0)
                        prs = m.blk_pairs[b]
                        psB = ppB.tile([P, F3 + H], f32, tag="psB", space="PSUM")
                        for i, (pr, sl) in enumerate(prs):
                            nc.tensor.matmul(psB[:, :Fl + H], lhsT=S[:, pr, :],
                                             rhs=wlhs[:, sl, :Fl + H],
                                             start=(i == 0), stop=(i == len(prs) - 1))
                        rec = tpool.tile([P, H], f32, tag="rec")
                        nc.vector.reciprocal(rec[:], psB[:, Fl:Fl + H])
                        if not l3:
                            hb = tpool.tile([P, F1], bf16, tag="hb")
                            nc.vector.tensor_tensor(
                                out=hb[:].rearrange("p (h c) -> p h c", h=H),
                                in0=psB[:, :Fl].rearrange("p (h c) -> p h c", h=H),
                                in1=rec[:, :, None].to_broadcast([P, H, Cl]), op=OP.mult)
                            nc.vector.tensor_tensor(out=hb[:], in0=hb[:],
                                                    in1=brep_sb[li][:], op=OP.add)
                            rp = tpool.tile([P, F1], bf16, tag="rp")
                            nc.scalar.activation(rp[:], hb[:], AF.Relu)
                            xm = tpool.tile([P, F1], bf16, tag="xm")
                            nc.vector.tensor_tensor(out=xm[:], in0=hb[:],
                                                    in1=zero64[:], op=OP.min)
                            ex = tpool.tile([P, F1], f32, tag="ex")
                            nc.scalar.activation(ex[:], xm[:], AF.Exp)
                            ho = tpool.tile([P, F1], bf16, tag="ho")
                            nc.vector.scalar_tensor_tensor(
                                out=ho[:], in0=ex[:], scalar=-1.0, in1=rp[:],
                                op0=OP.add, op1=OP.add)
                            # next layer phase A folded in: transpose -> matmul
                            psT = ppT.tile([F1, P], bf16, tag="psT", space="PSUM")
                            nc.tensor.transpose(psT[:, :mm], ho[:mm, :], ident[:mm, :mm])
                            hTn = tpool.tile([F1, P], bf16, tag="hTn")
                            nc.scalar.copy(hTn[:, :mm], psT[:, :mm])
                            psA = ppA.tile([P, 2 * F3], f32, tag="psA", space="PSUM")
                            nc.tensor.matmul(psA[:mm, :FEn], lhsT=hTn[:, :mm],
                                             rhs=W_sb[li + 1][:], start=True, stop=True)
                            nc.scalar.copy(xlrn[:mm, ib, :FEn], psA[:mm, :FEn])
                        else:
                            o3 = tpool.tile([P, F3], f32, tag="o3")
                            nc.vector.tensor_tensor(
                                out=o3[:].rearrange("p (h c) -> p h c", h=H),
                                in0=psB[:, :Fl].rearrange("p (h c) -> p h c", h=H),
                                in1=rec[:, :, None].to_broadcast([P, H, Cl]), op=OP.mult)
                            m01 = tpool.tile([P, C3], f32, tag="m01")
                            nc.vector.tensor_tensor(out=m01[:], in0=o3[:, 0:C3],
                                                    in1=o3[:, C3:2 * C3], op=OP.add)
                            m23 = tpool.tile([P, C3], f32, tag="m23")
                            nc.vector.tensor_tensor(out=m23[:], in0=o3[:, 2 * C3:3 * C3],
                                                    in1=o3[:, 3 * C3:4 * C3], op=OP.add)
                            ms = tpool.tile([P, C3], f32, tag="ms")
                            nc.vector.tensor_tensor(out=ms[:], in0=m01[:], in1=m23[:], op=OP.add)
                            of = tpool.tile([P, C3], f32, tag="of")
                            nc.vector.scalar_tensor_tensor(
                                out=of[:], in0=ms[:], scalar=0.25, in1=brep_sb[2][:],
                                op0=OP.mult, op1=OP.add)
                            nc.sync.dma_start(out_d[n0:n0 + mm, :], of[:mm, :])
                    # superblock-batched next-layer table writes
                    if not l3:
                        nfull = nb if b0 + nb * P <= SHARD else nb - 1
                        if nfull:
                            nc.sync.dma_start(
                                xlr_sh[li + 1][b0:b0 + nfull * P, :]
                                .rearrange("(j p) f -> p j f", p=P),
                                xlrn[:, :nfull, :FEn])
                            if li == 1:
                                nc.sync.dma_start(
                                    xl3_sh[b0:b0 + nfull * P, :]
                                    .rearrange("(j p) f -> p j f", p=P),
                                    xlrn[:, :nfull, :F3])
                        if nfull < nb:
                            mm = SHARD - (bs[0] + nfull) * P
                            nc.sync.dma_start(
                                xlr_sh[li + 1][(bs[0] + nfull) * P:SHARD, :],
                                xlrn[:mm, nfull, :FEn])
                            if li == 1:
                                nc.sync.dma_start(
                                    xl3_sh[(bs[0] + nfull) * P:SHARD, :],
                                    xlrn[:mm, nfull, :F3])

    nc.compile()
    return nc


def _prep_inputs(x, edge_index, Ws, atts):
    m = _preprocess(edge_index)
    ident = np.eye(P, dtype=np.float32).astype(BF16)
    iota = np.broadcast_to(np.arange(P, dtype=np.float32), (P, m.NPRmax, P)) \
        .reshape(P, m.NPRmax * P).astype(BF16).copy()
    common = {"ident": ident, "iota": iota}
    for li, ((Wl, Wr), a) in enumerate(zip(Ws, atts)):
        Fl = Wl.shape[1]
        common[f"W{li + 1}"] = np.concatenate([Wl, Wr], axis=1).astype(BF16)
        a_flat = np.asarray(a).reshape(Fl).astype(np.float32)
        common[f"arep{li + 1}"] = np.broadcast_to(a_flat, (P, m.SLOTmax, Fl)) \
            .reshape(P, m.SLOTmax * Fl).astype(BF16).copy()
    in_maps = []
    for c in range(NCORES):
        d = dict(common)
        d["xT"] = x[c * SHARD:(c + 1) * SHARD].T.astype(BF16).copy()
        d["srcw"] = m.src_w[c]
        d["dstw"] = m.dst_w[c]
        d["dstrel"] = m.rel_pm[c]
        in_maps.append(d)
    return in_maps, m


def kernel(x, edge_index, W1l, W1r, a1, b1, W2l, W2r, a2, b2, W3l, W3r, a3, b3,
           _trace=False):
    from concourse.bass_utils import run_bass_kernel_spmd

    x = np.asarray(x, dtype=np.float32)
    edge_index = np.asarray(edge_index, dtype=np.int32)
    in_maps, m = _prep_inputs(
        x, edge_index,
        [(np.asarray(W1l), np.asarray(W1r)), (np.asarray(W2l), np.asarray(W2r)),
         (np.asarray(W3l), np.asarray(W3r))],
        [a1, a2, a3])
    for c in range(NCORES):
        in_maps[c]["brep1"] = np.broadcast_to(np.asarray(b1, np.float32), (P, F1)).astype(BF16).copy()
        in_maps[c]["brep2"] = np.broadcast_to(np.asarray(b2, np.float32), (P, F1)).astype(BF16).copy()
        in_maps[c]["brep3"] = np.broadcast_to(np.asarray(b3, np.float32), (P, C3)).astype(np.float32).copy()

    key = (m.Gtot, m.Ptot, tuple(m.sb_g), tuple(m.sb_p),
           tuple(tuple(o) for o in m.sb_cls_off))
    if key not in _cache:
        _cache.clear()
        _cache[key] = _build_program(m)
    nc = _cache[key]

    res = run_bass_kernel_spmd(nc, in_maps, core_ids=list(range(NCORES)),
                               trace=_trace)
    out = np.concatenate([res.results[c]["out_shard"] for c in range(NCORES)], axis=0)
    kernel._last_result = res
    return out


# revision 10
# speedup vs baseline: 1.2527x; 1.2527x over previous
"""3-layer GATv2 (PyG GATv2Conv semantics) on 8 Trainium2 NeuronCores.

Distribution: nodes sharded 12500/core; edges (excl. appended self-loops)
partitioned by dst core, packed per (superblock, src%4 class) into 128-edge
gather groups that may straddle the superblock's dst blocks.

Layer 1 phase A ([xl|xr] = x @ [Wl|Wr]) runs chunked (14 blocks per DMA);
layers 2/3 fold their phase A into the previous layer's per-block tail (the
ELU output hTn is already feature-major = the matmul lhsT), so between
layers only the AllGather remains. Layer 3 all-gathers an xl-only table
(half the bytes).

Phase B per superblock: S matrices (is_equal against host-precomputed
block-shifted rel values, one per (slot, dst-block) pair), dst gathers
(local rows, 4 quarter calls), dense self-loop slots (one batched
dma_start), src gathers (4 src%4 class calls) -- all gathers spread across
the 4 SWDGE queues (each runs on its own Q7 core pair, pipelining ~4x).
z = xl[src]+xr[dst], lrelu on the Scalar engine (Prelu alpha=0.2, same act
table as Exp), score = reduce(z*a), w = exp(score), wlhs = [w*xg | w].
Per dst block: matmul S_pair.T @ wlhs accumulates weighted sums +
denominators in PSUM; divide, bias, ELU (min via tensor_tensor against a
zero tile -- tensor_scalar would block SWDGE on the shared SBUF port),
transpose, next-layer matmul, superblock-batched table writes. Layer 3:
divide then head-mean, f32 shard output, host concat.
"""
import sys
sys.path.insert(0, "/opt/trn_rl_repo")
import numpy as np
import ml_dtypes

N = 100000
E = 800000
NCORES = 8
SHARD = N // NCORES        # 12500
P = 128
NBLK = (SHARD + P - 1) // P  # 98
SB = 4                      # node blocks per superblock
CH = 14                     # layer-1 phase A blocks per chunk (98 = 7*14)
FIN = 64
H = 4
C1, C3 = 16, 32
F1 = H * C1                # 64
F3 = H * C3                # 128
NEG_SLOPE = 0.2
NQ = 4                      # SWDGE queues (Q7 core pairs)

BF16 = ml_dtypes.bfloat16

_cache = {}


class Meta:
    pass


def _preprocess(edge_index):
    """Sort real edges by dst; per (core, superblock, class) pack into
    128-edge groups (may straddle dst blocks). Appended self-loops become
    per-block dense slots. Pair list (slot, block) is the union over cores
    so the program is SPMD-uniform; a core without edges in a pair gets an
    all-miss rel row (S rows zero)."""
    src = edge_index[0]
    dst = edge_index[1]
    order = np.argsort(dst, kind="stable")
    src_s = src[order].astype(np.int64)
    dst_s = dst[order].astype(np.int64)

    core = dst_s // SHARD
    blk = (dst_s - core * SHARD) // P
    key = core * NBLK + blk
    cnt = np.bincount(key, minlength=NCORES * NBLK).reshape(NCORES, NBLK)
    starts = np.concatenate([[0], np.cumsum(cnt.reshape(-1))])

    NCLS = 4
    m = Meta()
    m.NCLS = NCLS
    m.NSB = (NBLK + SB - 1) // SB
    m.sb_blocks = [list(range(s * SB, min(NBLK, (s + 1) * SB))) for s in range(m.NSB)]

    # per (core, sb, class): concatenated (src, sb-rel dst) runs
    runs = {}
    gcnt = np.zeros((NCORES, m.NSB, NCLS), np.int64)
    for c in range(NCORES):
        for s, bs in enumerate(m.sb_blocks):
            sr_all, rel_all = [], []
            for b in bs:
                i = c * NBLK + b
                s0, e0 = starts[i], starts[i + 1]
                sr_all.append(src_s[s0:e0])
                rel_all.append(dst_s[s0:e0] - c * SHARD - bs[0] * P)
            sr_all = np.concatenate(sr_all)
            rel_all = np.concatenate(rel_all)
            cls = sr_all % NCLS
            for r in range(NCLS):
                sel = cls == r
                runs[(c, s, r)] = (sr_all[sel], rel_all[sel])
                gcnt[c, s, r] = sel.sum()
    Gsb = np.maximum(1, -(-gcnt.max(axis=0) // P))   # [NSB, NCLS] groups

    m.sb_cls_off = []   # per sb: class group offsets [o0..o4]
    m.sb_g = []         # per sb: gather slots
    m.sb_s = []         # per sb: total slots (gather + dense)
    for s, bs in enumerate(m.sb_blocks):
        offs = [0]
        for r in range(NCLS):
            offs.append(offs[-1] + int(Gsb[s, r]))
        m.sb_cls_off.append(offs)
        m.sb_g.append(offs[-1])
        m.sb_s.append(offs[-1] + len(bs))
    m.SLOTmax = max(m.sb_s)
    m.Gtot = sum(m.sb_g)
    sb_goff = np.concatenate([[0], np.cumsum(m.sb_g)])
    m.sb_goff = [int(v) for v in sb_goff]

    # fill per-core gather idx arrays + per-slot sb-rel dst (BIGNEG pad)
    BIGNEG = -16000.0
    src_idx = np.zeros((NCORES, m.Gtot * P), np.int16)
    dst_idx = np.zeros((NCORES, m.Gtot * P), np.int16)
    slot_rel = np.full((NCORES, m.Gtot, P), BIGNEG, np.float32)
    for c in range(NCORES):
        for s in range(m.NSB):
            gbase = m.sb_goff[s]
            offs = m.sb_cls_off[s]
            for r in range(NCLS):
                sr, rel = runs[(c, s, r)]
                o = (gbase + offs[r]) * P
                src_idx[c, o:o + len(sr)] = sr // NCLS
                dst_idx[c, o:o + len(rel)] = rel + m.sb_blocks[s][0] * P
                fl = slot_rel[c, gbase + offs[r]:gbase + offs[r + 1]].reshape(-1)
                fl[:len(rel)] = rel
    # pairs: union over cores of (slot, block_j) overlaps + dense pairs
    m.sb_pairs = []     # per sb: list of (slot, jj)
    m.blk_pairs = [[] for _ in range(NBLK)]  # per block: (pair_idx, slot_idx) sb-local
    m.sb_p = []         # per sb: pair count
    for s, bs in enumerate(m.sb_blocks):
        gbase = m.sb_goff[s]
        pairs = []
        for sl in range(m.sb_g[s]):
            jset = set()
            for c in range(NCORES):
                rels = slot_rel[c, gbase + sl]
                valid = rels > BIGNEG
                if valid.any():
                    jset |= set((rels[valid].astype(np.int64) // P).tolist())
            for jj in sorted(jset):
                pairs.append((sl, jj))
        for i, b in enumerate(bs):
            pairs.append((m.sb_g[s] + i, i))   # dense slot pair
        m.sb_pairs.append(pairs)
        m.sb_p.append(len(pairs))
        for pi, (sl, jj) in enumerate(pairs):
            m.blk_pairs[bs[0] + jj].append((pi, sl))
    m.NPRmax = max(m.sb_p)
    m.Ptot = sum(m.sb_p)
    sb_poff = np.concatenate([[0], np.cumsum(m.sb_p)])
    m.sb_poff = [int(v) for v in sb_poff]

    # per-pair shifted rel values (the S-build comparison targets)
    rel_pairs = np.full((NCORES, m.Ptot, P), -1.0, np.float32)
    for s, bs in enumerate(m.sb_blocks):
        gbase = m.sb_goff[s]
        pbase = m.sb_poff[s]
        for pi, (sl, jj) in enumerate(m.sb_pairs[s]):
            if sl < m.sb_g[s]:
                v = slot_rel[:, gbase + sl] - jj * P    # [NCORES, P]
                rel_pairs[:, pbase + pi] = np.where(v > BIGNEG / 2, v, -1.0)
            else:
                rel_pairs[:, pbase + pi] = np.arange(P)[None, :]

    # Wrap to dma_gather idx layout: flat[k] -> wrapped[k % 16, k // 16],
    # replicated to all 8 16-partition groups (each Q7 core pair reads its
    # own partitions, so all 4 SWDGE queues see the same idx stream).
    def wrap(a):
        w = a.reshape(NCORES, m.Gtot * P // 16, 16).transpose(0, 2, 1)
        return np.tile(w, (1, 8, 1)).copy()

    m.src_w = wrap(src_idx)
    m.dst_w = wrap(dst_idx)
    m.rel_pm = rel_pairs.transpose(0, 2, 1).astype(BF16).copy()  # [NC, P, Ptot]
    return m


def _build_program(m):
    import os
    import concourse.bass as bass
    import concourse.bacc as bacc
    import concourse.tile as tile
    from concourse import mybir, library_config

    STAGE = int(os.environ.get("BK_STAGE", "99"))
    NLAYER = int(os.environ.get("BK_NLAYER", "3"))
    SP = bool(int(os.environ.get("BK_SP", "0")))   # single_packet experiment

    bf16, f32, i16 = mybir.dt.bfloat16, mybir.dt.float32, mybir.dt.int16
    AF = mybir.ActivationFunctionType
    OP = mybir.AluOpType
    X = mybir.AxisListType.X

    nc = bacc.Bacc("TRN2", target_bir_lowering=False, num_swdge_queues=NQ)

    WI = m.Gtot * P // 16
    xT_d = nc.dram_tensor("xT", [FIN, SHARD], bf16, kind="ExternalInput")
    srcw_d = nc.dram_tensor("srcw", [P, WI], i16, kind="ExternalInput")
    dstw_d = nc.dram_tensor("dstw", [P, WI], i16, kind="ExternalInput")
    dstrel_d = nc.dram_tensor("dstrel", [P, m.Ptot], bf16, kind="ExternalInput")
    iota_d = nc.dram_tensor("iota", [P, m.NPRmax * P], bf16, kind="ExternalInput")
    W_d = [nc.dram_tensor(f"W{l}", [FIN, 2 * (F1 if l < 3 else F3)], bf16, kind="ExternalInput") for l in (1, 2, 3)]
    arep_d = [nc.dram_tensor(f"arep{l}", [P, F1 if l < 3 else F3], bf16, kind="ExternalInput") for l in (1, 2, 3)]
    brep_d = [nc.dram_tensor("brep1", [P, F1], bf16, kind="ExternalInput"),
              nc.dram_tensor("brep2", [P, F1], bf16, kind="ExternalInput"),
              nc.dram_tensor("brep3", [P, C3], f32, kind="ExternalInput")]
    ident_d = nc.dram_tensor("ident", [P, P], bf16, kind="ExternalInput")
    out_d = nc.dram_tensor("out_shard", [SHARD, C3], f32, kind="ExternalOutput")

    def internal(name, shape, dt, shared=False):
        return nc.dram_tensor(name, shape, dt, kind="Internal",
                              addr_space="Shared" if shared else "Local")

    xlr_sh = [internal(f"xlrsh{l}", [SHARD, 2 * (F1 if l < 3 else F3)], bf16) for l in (1, 2, 3)]
    xl3_sh = internal("xl3sh", [SHARD, F3], bf16)
    xlr_cc = [internal(f"xlrcc{l}", [N, 2 * F1], bf16, shared=True) for l in (1, 2)]
    xl3_cc = internal("xl3cc", [N, F3], bf16, shared=True)

    RG = [list(range(NCORES))]

    with tile.TileContext(nc) as tc:
        nc.gpsimd.load_library(library_config.mlp)
        with tc.tile_pool(name="const", bufs=1) as cpool, \
             tc.tile_pool(name="work", bufs=2) as wpool, \
             tc.tile_pool(name="gath", bufs=3) as gpool, \
             tc.tile_pool(name="mmA", bufs=3) as apool, \
             tc.tile_pool(name="tail", bufs=3) as tpool, \
             tc.tile_pool(name="psA", bufs=2, space="PSUM") as ppA, \
             tc.tile_pool(name="psB", bufs=4, space="PSUM") as ppB, \
             tc.tile_pool(name="psT", bufs=2, space="PSUM") as ppT:

            srcw = cpool.tile([P, WI], i16)
            dstw = cpool.tile([P, WI], i16)
            dstrel = cpool.tile([P, m.Ptot], bf16)
            iota = cpool.tile([P, m.NPRmax * P], bf16)
            ident = cpool.tile([P, P], bf16)
            for t, d in [(srcw, srcw_d), (dstw, dstw_d), (dstrel, dstrel_d),
                         (iota, iota_d), (ident, ident_d)]:
                nc.sync.dma_start(t[:], d[:])
            zero64 = cpool.tile([P, F1], bf16, tag="zero64")
            nc.vector.memset(zero64[:], 0.0)
            W_sb, arep_sb, brep_sb = [], [], []
            for li in range(3):
                Fl = F1 if li < 2 else F3
                w = cpool.tile([FIN, 2 * Fl], bf16, tag=f"W{li}")
                nc.sync.dma_start(w[:], W_d[li][:])
                W_sb.append(w)
                a = cpool.tile([P, Fl], bf16, tag=f"arep{li}")
                nc.sync.dma_start(a[:], arep_d[li][:])
                arep_sb.append(a)
                b = cpool.tile([P, F1 if li < 2 else C3], bf16 if li < 2 else f32, tag=f"brep{li}")
                nc.sync.dma_start(b[:], brep_d[li][:])
                brep_sb.append(b)

            # ---- layer 1 phase A: x @ [W1l|W1r], chunked ----
            FE0 = 2 * F1
            for c0 in range(0, NBLK, CH):
                nblk = min(CH, NBLK - c0)
                n0 = c0 * P
                ncols = min(SHARD - n0, nblk * P)
                hTc = apool.tile([FIN, CH * P], bf16, tag="hTc")
                nc.sync.dma_start(hTc[:, :ncols], xT_d[:, n0:n0 + ncols])
                xlrc = apool.tile([P, CH, FE0], bf16, tag="xlrc")
                for j in range(nblk):
                    mm = min(P, SHARD - (c0 + j) * P)
                    psA = ppA.tile([P, 2 * F3], f32, tag="psA", space="PSUM")
                    nc.tensor.matmul(psA[:mm, :FE0], lhsT=hTc[:, j * P:j * P + mm],
                                     rhs=W_sb[0][:], start=True, stop=True)
                    nc.scalar.copy(xlrc[:mm, j, :], psA[:mm, :FE0])
                nfull = nblk if (c0 + nblk) * P <= SHARD else nblk - 1
                if nfull:
                    nc.sync.dma_start(
                        xlr_sh[0][n0:n0 + nfull * P, :]
                        .rearrange("(j p) f -> p j f", p=P),
                        xlrc[:, :nfull, :])
                if nfull < nblk:
                    mm = SHARD - (c0 + nfull) * P
                    nc.sync.dma_start(xlr_sh[0][(c0 + nfull) * P:SHARD, :],
                                      xlrc[:mm, nfull, :])

            for li in range(3):
                l3 = (li == 2)
                Fl = F3 if l3 else F1
                Cl = C3 if l3 else C1
                FE = 2 * Fl            # row width of xlr tensors
                FEn = 2 * (F3 if li + 1 >= 2 else F1)   # next layer row width

                if STAGE >= 2:
                    if not l3:
                        nc.gpsimd.collective_compute(
                            "AllGather", mybir.AluOpType.bypass, replica_groups=RG,
                            ins=[xlr_sh[li][:]], outs=[xlr_cc[li][:]])
                        gtab = xlr_cc[li]
                    else:
                        nc.gpsimd.collective_compute(
                            "AllGather", mybir.AluOpType.bypass, replica_groups=RG,
                            ins=[xl3_sh[:]], outs=[xl3_cc[:]])
                        gtab = xl3_cc
                else:
                    gtab = xlr_sh[li] if not l3 else xl3_sh

                # gather source views: 4 src classes (src%4, idx=src//4)
                if not l3:
                    src_tabs = [gtab[r::4, :] for r in range(m.NCLS)]
                    dst_tab = xlr_sh[li][:]                   # local rows, use xr half
                    GELEM, GSTEP = FE, 4 * FE
                    DELEM, DSTEP = FE, FE
                else:
                    src_tabs = [gtab[r::4, :] for r in range(m.NCLS)]
                    dst_tab = xlr_sh[li][:, F3:2 * F3]        # xr3 half
                    GELEM, GSTEP = F3, 4 * F3
                    DELEM, DSTEP = F3, FE

                # ---- phase B ----
                if STAGE < 3 or li >= NLAYER:
                    continue
                for s in range(m.NSB):
                    SG = m.sb_g[s]       # gather slots
                    ST = m.sb_s[s]       # total slots (gather + dense)
                    NPR = m.sb_p[s]      # pairs
                    go = m.sb_goff[s]
                    po = m.sb_poff[s]
                    wo = go * P // 16
                    bs = m.sb_blocks[s]
                    nb = len(bs)
                    b0 = bs[0] * P
                    nbf = nb if b0 + nb * P <= SHARD else nb - 1  # full dense blocks
                    # S matrices first (no gather dependency; overlaps AG)
                    S = wpool.tile([P, m.NPRmax, P], bf16, tag="S")
                    nc.vector.tensor_tensor(
                        out=S[:, :NPR, :],
                        in0=iota[:, :NPR * P].rearrange("p (g n) -> p g n", g=NPR),
                        in1=dstrel[:, po:po + NPR].to_broadcast([P, NPR, P]),
                        op=OP.is_equal)
                    xg = gpool.tile([P, m.SLOTmax, F3 if l3 else FE], bf16, tag="xg")
                    xrg = gpool.tile([P, m.SLOTmax, F3 if l3 else FE], bf16, tag="xrg")
                    # dst gather (local table): split into NQ quarter-calls
                    qcuts = [SG * q // NQ for q in range(NQ + 1)]
                    for q in range(NQ):
                        qs, qe = qcuts[q], qcuts[q + 1]
                        if qe == qs:
                            continue
                        nc.gpsimd.dma_gather(
                            out_ap=xrg[:, qs:qe, :DELEM], in_ap=dst_tab,
                            idxs_ap=dstw[:, wo + qs * 8:wo + qe * 8],
                            num_idxs=(qe - qs) * P, num_idxs_reg=(qe - qs) * P,
                            elem_size=DELEM, elem_step=DSTEP,
                            single_packet=SP, queue_num=q)
                    # dense self-loop slots: sb's own rows, batched DMAs
                    if not l3:
                        dsrc = [(xg, slice(0, FE), slice(0, FE)),
                                (xrg, slice(0, FE), slice(0, FE))]
                    else:
                        dsrc = [(xg, slice(0, F3), slice(0, F3)),
                                (xrg, slice(0, F3), slice(F3, 2 * F3))]
                    for tl, tsl, csl in dsrc:
                        if nbf:
                            nc.sync.dma_start(
                                tl[:, SG:SG + nbf, tsl],
                                xlr_sh[li][b0:b0 + nbf * P, csl]
                                .rearrange("(j p) f -> p j f", p=P))
                        if nbf < nb:
                            mm = SHARD - (bs[0] + nbf) * P
                            nc.sync.dma_start(
                                tl[:mm, SG + nbf, tsl],
                                xlr_sh[li][(bs[0] + nbf) * P:SHARD, csl])
                    # src gathers (need the all-gathered table): class r -> queue r
                    offs = m.sb_cls_off[s]
                    for r in range(m.NCLS):
                        nr = (offs[r + 1] - offs[r]) * P
                        if nr == 0:
                            continue
                        nc.gpsimd.dma_gather(
                            out_ap=xg[:, offs[r]:offs[r + 1], :GELEM], in_ap=src_tabs[r],
                            idxs_ap=srcw[:, wo + offs[r] * 8:wo + offs[r + 1] * 8],
                            num_idxs=nr, num_idxs_reg=nr, elem_size=GELEM,
                            elem_step=GSTEP, single_packet=SP, queue_num=r)
                    if STAGE < 4:
                        continue
                    xgv = xg[:, :ST, :Fl]                      # xl[src]
                    xrv = xrg[:, :ST, Fl:FE] if not l3 else xrg[:, :ST, :Fl]  # xr[dst]
                    # z lives in the unused xl[dst] half of xrg (l1/2) or
                    # overwrites xr in place (l3): z = xg + xr, lrelu (ACT)
                    zv = xrg[:, :ST, :Fl]
                    nc.vector.tensor_tensor(out=zv, in0=xgv, in1=xrv, op=OP.add)
                    nc.scalar.activation(zv, zv, AF.Prelu, alpha=NEG_SLOPE)
                    nc.vector.tensor_tensor(
                        out=zv, in0=zv,
                        in1=arep_sb[li][:, None, :].to_broadcast([P, ST, Fl]),
                        op=OP.mult)
                    score = wpool.tile([P, m.SLOTmax * H], f32, tag="score")
                    nc.vector.tensor_reduce(
                        out=score[:, :ST * H],
                        in_=zv.rearrange("p g (h c) -> p g h c", h=H),
                        axis=X, op=OP.add)
                    wlhs = wpool.tile([P, m.SLOTmax, F3 + H], bf16, tag="wlhs")
                    nc.scalar.activation(
                        wlhs[:, :ST, Fl:Fl + H],
                        score[:, :ST * H].rearrange("p (g h) -> p g h", g=ST),
                        AF.Exp)
                    nc.vector.tensor_tensor(
                        out=wlhs[:, :ST, :Fl].rearrange("p g (h c) -> p g h c", h=H),
                        in0=xgv.rearrange("p g (h c) -> p g h c", h=H),
                        in1=wlhs[:, :ST, Fl:Fl + H, None].to_broadcast([P, ST, H, Cl]),
                        op=OP.mult)

                    if STAGE < 5:
                        continue
                    xlrn = None
                    if not l3:
                        xlrn = apool.tile([P, SB, 2 * F3], bf16, tag="xlrn")
                    for ib, b in enumerate(bs):
                        n0 = b * P
                        mm = min(P, SHARD - n0)
                        prs = m.blk_pairs[b]
                        psB = ppB.tile([P, F3 + H], f32, tag="psB", space="PSUM")
                        for i, (pr, sl) in enumerate(prs):
                            nc.tensor.matmul(psB[:, :Fl + H], lhsT=S[:, pr, :],
                                             rhs=wlhs[:, sl, :Fl + H],
                                             start=(i == 0), stop=(i == len(prs) - 1))
                        rec = tpool.tile([P, H], f32, tag="rec")
                        nc.vector.reciprocal(rec[:], psB[:, Fl:Fl + H])
                        if not l3:
                            hb = tpool.tile([P, F1], bf16, tag="hb")
                            nc.vector.tensor_tensor(
                                out=hb[:].rearrange("p (h c) -> p h c", h=H),
                                in0=psB[:, :Fl].rearrange("p (h c) -> p h c", h=H),
                                in1=rec[:, :, None].to_broadcast([P, H, Cl]), op=OP.mult)
                            nc.vector.tensor_tensor(out=hb[:], in0=hb[:],
                                                    in1=brep_sb[li][:], op=OP.add)
                            rp = tpool.tile([P, F1], bf16, tag="rp")
                            nc.scalar.activation(rp[:], hb[:], AF.Relu)
                            xm = tpool.tile([P, F1], bf16, tag="xm")
                            nc.vector.tensor_tensor(out=xm[:], in0=hb[:],
                                                    in1=zero64[:], op=OP.min)
                            ex = tpool.tile([P, F1], f32, tag="ex")
                            nc.scalar.activation(ex[:], xm[:], AF.Exp)
                            ho = tpool.tile([P, F1], bf16, tag="ho")
                            nc.vector.scalar_tensor_tensor(
                                out=ho[:], in0=ex[:], scalar=-1.0, in1=rp[:],
                                op0=OP.add, op1=OP.add)
                            # next layer phase A folded in: transpose -> matmul
                            psT = ppT.tile([F1, P], bf16, tag="psT", space="PSUM")
                            nc.tensor.transpose(psT[:, :mm], ho[:mm, :], ident[:mm, :mm])
                            hTn = tpool.tile([F1, P], bf16, tag="hTn")
                            nc.scalar.copy(hTn[:, :mm], psT[:, :mm])
                            psA = ppA.tile([P, 2 * F3], f32, tag="psA", space="PSUM")
                            nc.tensor.matmul(psA[:mm, :FEn], lhsT=hTn[:, :mm],
                                             rhs=W_sb[li + 1][:], start=True, stop=True)
                            nc.scalar.copy(xlrn[:mm, ib, :FEn], psA[:mm, :FEn])
                        else:
                            o3 = tpool.tile([P, F3], f32, tag="o3")
                            nc.vector.tensor_tensor(
                                out=o3[:].rearrange("p (h c) -> p h c", h=H),
                                in0=psB[:, :Fl].rearrange("p (h c) -> p h c", h=H),
                                in1=rec[:, :, None].to_broadcast([P, H, Cl]), op=OP.mult)
                            m01 = tpool.tile([P, C3], f32, tag="m01")
                            nc.vector.tensor_tensor(out=m01[:], in0=o3[:, 0:C3],
                                                    in1=o3[:, C3:2 * C3], op=OP.add)
                            m23 = tpool.tile([P, C3], f32, tag="m23")
                            nc.vector.tensor_tensor(out=m23[:], in0=o3[:, 2 * C3:3 * C3],
                                                    in1=o3[:, 3 * C3:4 * C3], op=OP.add)
                            ms = tpool.tile([P, C3], f32, tag="ms")
                            nc.vector.tensor_tensor(out=ms[:], in0=m01[:], in1=m23[:], op=OP.add)
                            of = tpool.tile([P, C3], f32, tag="of")
                            nc.vector.scalar_tensor_tensor(
                                out=of[:], in0=ms[:], scalar=0.25, in1=brep_sb[2][:],
                                op0=OP.mult, op1=OP.add)
                            nc.sync.dma_start(out_d[n0:n0 + mm, :], of[:mm, :])
                    # superblock-batched next-layer table writes
                    if not l3:
                        nfull = nb if b0 + nb * P <= SHARD else nb - 1
                        if nfull:
                            nc.sync.dma_start(
                                xlr_sh[li + 1][b0:b0 + nfull * P, :]
                                .rearrange("(j p) f -> p j f", p=P),
                                xlrn[:, :nfull, :FEn])
                            if li == 1:
                                nc.sync.dma_start(
                                    xl3_sh[b0:b0 + nfull * P, :]
                                    .rearrange("(j p) f -> p j f", p=P),
                                    xlrn[:, :nfull, :F3])
                        if nfull < nb:
                            mm = SHARD - (bs[0] + nfull) * P
                            nc.sync.dma_start(
                                xlr_sh[li + 1][(bs[0] + nfull) * P:SHARD, :],
                                xlrn[:mm, nfull, :FEn])
                            if li == 1:
                                nc.sync.dma_start(
                                    xl3_sh[(bs[0] + nfull) * P:SHARD, :],
                                    xlrn[:mm, nfull, :F3])

    nc.compile()
    return nc


def _prep_inputs(x, edge_index, Ws, atts):
    m = _preprocess(edge_index)
    ident = np.eye(P, dtype=np.float32).astype(BF16)
    iota = np.broadcast_to(np.arange(P, dtype=np.float32), (P, m.NPRmax, P)) \
        .reshape(P, m.NPRmax * P).astype(BF16).copy()
    common = {"ident": ident, "iota": iota}
    for li, ((Wl, Wr), a) in enumerate(zip(Ws, atts)):
        Fl = Wl.shape[1]
        common[f"W{li + 1}"] = np.concatenate([Wl, Wr], axis=1).astype(BF16)
        a_flat = np.asarray(a).reshape(Fl).astype(np.float32)
        common[f"arep{li + 1}"] = np.broadcast_to(a_flat, (P, Fl)).astype(BF16).copy()
    in_maps = []
    for c in range(NCORES):
        d = dict(common)
        d["xT"] = x[c * SHARD:(c + 1) * SHARD].T.astype(BF16).copy()
        d["srcw"] = m.src_w[c]
        d["dstw"] = m.dst_w[c]
        d["dstrel"] = m.rel_pm[c]
        in_maps.append(d)
    return in_maps, m


def kernel(x, edge_index, W1l, W1r, a1, b1, W2l, W2r, a2, b2, W3l, W3r, a3, b3,
           _trace=False):
    from concourse.bass_utils import run_bass_kernel_spmd

    x = np.asarray(x, dtype=np.float32)
    edge_index = np.asarray(edge_index, dtype=np.int32)
    in_maps, m = _prep_inputs(
        x, edge_index,
        [(np.asarray(W1l), np.asarray(W1r)), (np.asarray(W2l), np.asarray(W2r)),
         (np.asarray(W3l), np.asarray(W3r))],
        [a1, a2, a3])
    for c in range(NCORES):
        in_maps[c]["brep1"] = np.broadcast_to(np.asarray(b1, np.float32), (P, F1)).astype(BF16).copy()
        in_maps[c]["brep2"] = np.broadcast_to(np.asarray(b2, np.float32), (P, F1)).astype(BF16).copy()
        in_maps[c]["brep3"] = np.broadcast_to(np.asarray(b3, np.float32), (P, C3)).astype(np.float32).copy()

    key = (m.Gtot, m.Ptot, tuple(m.sb_g), tuple(m.sb_p),
           tuple(tuple(o) for o in m.sb_cls_off))
    if key not in _cache:
        _cache.clear()
        _cache[key] = _build_program(m)
    nc = _cache[key]

    res = run_bass_kernel_spmd(nc, in_maps, core_ids=list(range(NCORES)),
                               trace=_trace)
    out = np.concatenate([res.results[c]["out_shard"] for c in range(NCORES)], axis=0)
    kernel._last_result = res
    return out
